# revision 2
# baseline (speedup 1.0000x reference)
"""Trainium2 Bass kernel for masked candidate-span attention (ragged_sequence).

Math (per char n):
  q = W @ x_n                         [128]
  s_v = pos_embed[v] . q  for all v   [96]   (dense: s = x @ (pos_embed@W)^T)
  scores over 9 candidates c are s[idx_c]; masked softmax over c collapses to
  v-space with multiplicities: w_v = cnt_v * exp(s_v - m), Z = sum w,
  ctx = (w/Z) @ pos_embed, where cnt_v = #{c : idx_c == v and mask_c}.
  Rows with no masked-in candidate or l >= seq_len output 0.

Sharding: pure data parallel over batch (2 batches per core x 8 cores).
"""
import os
import sys

import numpy as np

sys.path.insert(0, "/opt/trn_rl_repo")
_HERE = os.path.dirname(os.path.abspath(__file__))
sys.path.insert(0, _HERE)

from contextlib import ExitStack

import concourse.bass as bass  # noqa: E402
import concourse.mybir as mybir  # noqa: E402
from concourse.tile import TileContext  # noqa: E402

# --- walrus workaround: cap sync waits per instruction ---------------------
import concourse.tile as _tile_mod  # noqa: E402
import bass_rust as _br  # noqa: E402
from concourse.vector_clock import ScopedClock  # noqa: E402


def _patched_drain_and_barrier(self, tick_clock, wait_clock):
    nc = self.nc
    probe = mybir.InstNoOp(name=nc.get_next_instruction_name(), ins=[], outs=[])
    probe.engine = mybir.EngineType.SP
    wait_clock.add_sem_waits(probe, ScopedClock({None: tick_clock.global_clock}))
    waits = list(probe.sync_info.on_wait)
    assert self.sems is not None
    by_num = {h.num: h for h in self.sems.allocated().values()}
    for w in waits:
        nc.sync.wait_ge(by_num[w.id], w.wait_value)
    nc.sync.drain()
    nc.all_engine_barrier()
    popped = nc._tile_sem_poison_stack.pop()
    assert popped is self._sem_poison
    nc.clear_and_free_semaphores(list(self.sems.allocated().values()))
    nc.all_engine_barrier()


_tile_mod.TileContext._drain_and_barrier = _patched_drain_and_barrier


def split_excess_waits(nc):
    for f in nc.m.functions:
        for bb in f.blocks:
            out = []
            changed = False
            for inst in bb.instructions:
                si = inst.sync_info
                waits = list(si.on_wait) if si is not None else []
                cap = 2 if isinstance(inst, _br.InstEventSemaphore) else 1
                if len(waits) > cap:
                    excess, keep = waits[:-cap], waits[-cap:]
                    for k in range(0, len(excess), 2):
                        ev = _br.InstEventSemaphore(
                            name=f"{inst.name}-wsplit{k}", ins=[], outs=[])
                        ev.engine = inst.engine
                        ev.sync_info = _br.SyncInfo(on_wait=excess[k:k + 2],
                                                    on_update=[])
                        out.append(ev)
                    inst.sync_info = _br.SyncInfo(on_wait=keep,
                                                  on_update=list(si.on_update))
                    changed = True
                out.append(inst)
            if changed:
                bb.instructions = out


# --- problem constants -----------------------------------------------------
B, L, C = 16, 4096, 9
DI, DO, V = 512, 128, 96
NCORES = 8
BLOC = B // NCORES          # batches per core
NLOC = BLOC * L             # chars per core (8192)
NTILE = NLOC // 128         # 64 char-tiles per core
NSUP = NTILE // 4           # 16 super-tiles (512 chars each)
BIG = 1.0e9

f32 = mybir.dt.float32
f32r = mybir.dt.float32r
f16 = mybir.dt.float16
bf16 = mybir.dt.bfloat16
i32 = mybir.dt.int32
i16 = mybir.dt.int16
u8 = mybir.dt.uint8
Alu = mybir.AluOpType
Act = mybir.ActivationFunctionType
Ax = mybir.AxisListType

USE_F32R = True


def _ap0(ap, free_count):
    """Broadcast a [P,1] AP along free dim with stride 0."""
    return bass.AP(ap.tensor, ap.offset, [ap.ap[0], [0, free_count]])


def build_kernel():
    nc = bass.Bass()
    mmdt = f32r if USE_F32R else f32
    mmdt_x = mmdt
    x_d = nc.declare_dram_parameter("x", [NLOC, DI], f32, isOutput=False)
    idx_d = nc.declare_dram_parameter("cand_idx", [NLOC, C], i32, isOutput=False)
    msk_d = nc.declare_dram_parameter("cand_mask", [NLOC, C], u8, isOutput=False)
    w_d = nc.declare_dram_parameter("W", [DO, DI], f32, isOutput=False)
    pos_d = nc.declare_dram_parameter("pos_embed", [V, DO], f32, isOutput=False)
    len_d = nc.declare_dram_parameter("word_seq_len", [1, BLOC], i32, isOutput=False)
    out_d = nc.declare_dram_parameter("out", [NLOC, DO], f32, isOutput=True)


    with TileContext(nc) as tc, ExitStack() as es:
        cpool = es.enter_context(tc.tile_pool(name="consts", bufs=1))
        # ---- constants ----
        # identity 128 (f32) for PE transposes
        io_r = cpool.tile([128, 128], i32)
        io_c = cpool.tile([128, 1], i32)
        nc.gpsimd.iota(io_r[:], pattern=[[1, 128]], base=0, channel_multiplier=0)
        nc.gpsimd.iota(io_c[:], pattern=[[0, 1]], base=0, channel_multiplier=1)
        io_rf = cpool.tile([128, 128], f32)
        io_cf = cpool.tile([128, 1], f32)
        nc.vector.tensor_copy(io_rf[:], io_r[:])
        nc.vector.tensor_copy(io_cf[:], io_c[:])
        ident128 = cpool.tile([128, 128], f32)
        nc.vector.tensor_scalar(out=ident128[:], in0=io_rf[:], scalar1=io_cf[:],
                                scalar2=None, op0=Alu.is_equal)
        ident96 = cpool.tile([96, 96], f32)
        nc.vector.tensor_scalar(out=ident96[:], in0=io_rf[0:96, 0:96],
                                scalar1=io_cf[0:96, :], scalar2=None,
                                op0=Alu.is_equal)
        ident128h = cpool.tile([128, 128], f16)
        nc.vector.tensor_copy(ident128h[:], ident128[:])

        # iota-rep [128, 864] bf16: value v at column v*9+c (for one-hot eq)
        iorep_i = cpool.tile([128, 864], i16)
        nc.gpsimd.iota(iorep_i[:], pattern=[[1, 96], [0, 9]], base=0,
                       channel_multiplier=0)
        iota_rep = cpool.tile([128, 864], f16)
        nc.vector.tensor_copy(iota_rep[:], iorep_i[:])

        # in-length mask [128, NTILE] f32 (1.0 where char < seq_len)
        iol_i = cpool.tile([128, NTILE], i32)
        nc.gpsimd.iota(iol_i[:], pattern=[[0, BLOC], [128, NTILE // BLOC]],
                       base=0, channel_multiplier=1)
        len_sb = cpool.tile([128, BLOC], i32)
        nc.sync.dma_start(out=len_sb[:],
                          in_=bass.AP(len_d[:].tensor, 0, [[0, 128], [1, BLOC]]))
        inlen = cpool.tile([128, NTILE], f32)
        tpb = NTILE // BLOC
        for b_ in range(BLOC):
            nc.vector.tensor_tensor(
                out=inlen[:, b_ * tpb:(b_ + 1) * tpb],
                in0=iol_i[:, b_ * tpb:(b_ + 1) * tpb],
                in1=_ap0(len_sb[:, b_:b_ + 1], tpb), op=Alu.is_lt)

        # ---- weights: MT chunks [128d, 96v] = (pos_embed @ W)^T ----
        w_sb = cpool.tile([128, DI], f32)
        nc.sync.dma_start(out=w_sb[:], in_=w_d[:])
        pos_sb = cpool.tile([96, 128], f32)
        nc.sync.dma_start(out=pos_sb[:], in_=pos_d[:])
        pos_r = cpool.tile([96, 128], mmdt)
        nc.vector.tensor_copy(pos_r[:], pos_sb[:])

        with tc.tile_pool(name="pre_psum", bufs=2, space="PSUM") as ppool:
            ps_pt = ppool.tile([128, 96], f32, tag="pp")
            nc.tensor.transpose(ps_pt[:], pos_sb[:], ident96[:])
            posT = cpool.tile([128, 96], f32)
            nc.vector.tensor_copy(posT[:], ps_pt[:])
            mt_sb = []
            for k in range(4):
                ps_mt = ppool.tile([128, 96], f32, tag="pp")
                nc.tensor.matmul(ps_mt[:], w_sb[:, k * 128:(k + 1) * 128],
                                 posT[:], start=True, stop=True)
                t = cpool.tile([128, 96], mmdt, tag=f"mt{k}")
                nc.vector.tensor_copy(t[:], ps_mt[:])
                mt_sb.append(t)

        # ---- pools ----
        xpool = es.enter_context(tc.tile_pool(name="x", bufs=3))
        xtpool = es.enter_context(tc.tile_pool(name="xt", bufs=8))
        spool = es.enter_context(tc.tile_pool(name="soft", bufs=4))
        opool = es.enter_context(tc.tile_pool(name="outp", bufs=3))
        ps_xt = es.enter_context(tc.tile_pool(name="ps_xt", bufs=2, space="PSUM"))
        ps_misc = es.enter_context(tc.tile_pool(name="ps_misc", bufs=4, space="PSUM"))
        ps_sm = es.enter_context(tc.tile_pool(name="ps_sm", bufs=2, space="PSUM"))

        for st in range(NSUP):
            n0 = st * 512
            # x super-tile [128, 4(j) * 512(d)]
            xs = xpool.tile([128, 4 * DI], f32, tag="xs")
            src = x_d[n0:n0 + 512, :].rearrange("(j p) d -> p j d", p=128)
            nc.sync.dma_start(out=xs[:].rearrange("p (j d) -> p j d", j=4),
                              in_=src)
            # transpose x -> xT chunks [128d, 512n]
            xt = []
            for k in range(4):
                pxt = ps_xt.tile([128, 512], f32, tag="pxt")
                for j in range(4):
                    nc.tensor.transpose(
                        pxt[:, j * 128:(j + 1) * 128],
                        xs[:, j * DI + k * 128: j * DI + (k + 1) * 128],
                        ident128[:])
                t = xtpool.tile([128, 512], mmdt, tag=f"xt{k}")
                nc.scalar.copy(t[:], pxt[:])
                xt.append(t)
            # scores s^T [96v, 512n]
            pst = ps_misc.tile([96, 512], f32, tag="pm")
            for k in range(4):
                nc.tensor.matmul(pst[:], mt_sb[k][:], xt[k][:],
                                 start=(k == 0), stop=(k == 3))
            st_sb = spool.tile([96, 512], f32, tag="st")
            nc.scalar.copy(st_sb[:], pst[:])
            # s char layout [128n, 4x96]
            ps_s = ps_misc.tile([128, 384], f32, tag="pm")
            for j in range(4):
                nc.tensor.transpose(ps_s[:, j * 96:(j + 1) * 96],
                                    st_sb[:, j * 128:(j + 1) * 128],
                                    ident96[:])

            # ---- candidate indices -> masked sentinel f16 ----
            idxt = spool.tile([128, 36], i32, tag="idxt")
            nc.sync.dma_start(
                out=idxt[:].rearrange("p (j c) -> p j c", j=4),
                in_=idx_d[n0:n0 + 512, :].rearrange("(j p) c -> p j c", p=128))
            mskt = spool.tile([128, 36], u8, tag="mskt")
            nc.sync.dma_start(
                out=mskt[:].rearrange("p (j c) -> p j c", j=4),
                in_=msk_d[n0:n0 + 512, :].rearrange("(j p) c -> p j c", p=128))
            # idxm = idx + (1-mask)*1000  (sentinel never matches 0..95)
            sent = spool.tile([128, 36], i32, tag="sent")
            nc.vector.tensor_scalar(out=sent[:], in0=mskt[:], scalar1=-1000,
                                    scalar2=1000, op0=Alu.mult, op1=Alu.add)
            idxm = spool.tile([128, 36], f16, tag="idxm")
            nc.vector.tensor_tensor(out=idxm[:], in0=idxt[:], in1=sent[:],
                                    op=Alu.add)

            # ---- cnt via one-hot expansion [128, 864] per sub-tile ----
            cnt = spool.tile([128, 384], f16, tag="cnt")
            with nc.allow_low_precision("cnt<=9 exact in f16"):
                # batched one-hot expansion for all 4 subtiles: [128, 4*864]
                eq = spool.tile([128, 3456], f16, tag="eq")
                eqv = eq[:].rearrange("p (j v c) -> p j v c", v=96, c=9)
                iota4 = bass.AP(iota_rep[:].tensor, iota_rep[:].offset,
                                [iota_rep[:].ap[0], [0, 4], [9, 96], [1, 9]])
                idx4 = bass.AP(idxm[:].tensor, idxm[:].offset,
                               [idxm[:].ap[0], [9, 4], [0, 96], [1, 9]])
                nc.vector.tensor_tensor(out=eqv, in0=iota4, in1=idx4,
                                        op=Alu.is_equal)
                s1 = spool.tile([128, 1536], f16, tag="tr_s1")
                s1v = s1[:].rearrange("p (j v c) -> p j v c", v=96, c=4)
                nc.vector.tensor_tensor(out=s1v, in0=eqv[:, :, :, 0:4],
                                        in1=eqv[:, :, :, 4:8], op=Alu.add)
                s2 = spool.tile([128, 768], f16, tag="tr_s2")
                s2v = s2[:].rearrange("p (j v c) -> p j v c", v=96, c=2)
                nc.vector.tensor_tensor(out=s2v, in0=s1v[:, :, :, 0:2],
                                        in1=s1v[:, :, :, 2:4], op=Alu.add)
                s3 = spool.tile([128, 384], f16, tag="tr_s3")
                s3v = s3[:].rearrange("p (j v c) -> p j v c", v=96, c=1)
                nc.vector.tensor_tensor(out=s3v, in0=s2v[:, :, :, 0:1],
                                        in1=s2v[:, :, :, 1:2], op=Alu.add)
                cntv = cnt[:].rearrange("p (j v c) -> p j v c", v=96, c=1)
                nc.vector.tensor_tensor(out=cntv, in0=s3v,
                                        in1=eqv[:, :, :, 8:9], op=Alu.add)

            # ---- softmax in char layout (no max subtraction needed:
            #      |s| <= ~65 so exp(s) stays in fp32 range) ----
            e = spool.tile([128, 384], f32, tag="e")
            nc.scalar.activation(out=e[:], in_=ps_s[:], func=Act.Exp,
                                 bias=0.0, scale=1.0)
            w = spool.tile([128, 384], f32, tag="w")
            nc.vector.tensor_tensor(out=w[:], in0=cnt[:], in1=e[:], op=Alu.mult)
            z = spool.tile([128, 4], f32, tag="z")
            nc.vector.tensor_reduce(out=z[:],
                                    in_=w[:].rearrange("p (j v) -> p j v", j=4),
                                    axis=Ax.X, op=Alu.add)
            zg = spool.tile([128, 4], f32, tag="zg")
            nc.vector.tensor_scalar(out=zg[:], in0=z[:], scalar1=1e-30,
                                    scalar2=None, op0=Alu.max)
            rz = spool.tile([128, 4], f32, tag="rz")
            nc.vector.reciprocal(rz[:], zg[:])
            rzf = spool.tile([128, 4], f32, tag="rzf")
            nc.vector.tensor_tensor(out=rzf[:], in0=rz[:],
                                    in1=inlen[:, st * 4:(st + 1) * 4],
                                    op=Alu.mult)
            # ---- ctx: transpose w (f32r), matmul with pos, normalize in
            #      the psum-drain copy ----
            outsb = opool.tile([128, 512], f32, tag="outsb")
            for j in range(4):
                pwt = ps_sm.tile([96, 128], f32, tag="psm")
                nc.tensor.transpose(pwt[:], w[:, j * 96:(j + 1) * 96],
                                    ident128[:])
                wt = spool.tile([96, 128], mmdt, tag=f"wt{j % 2}")
                nc.scalar.copy(wt[:], pwt[:])
                pctx = ps_sm.tile([128, 128], f32, tag="psm")
                nc.tensor.matmul(pctx[:], wt[:], pos_r[:], start=True, stop=True)
                if j % 2 == 0:
                    nc.vector.tensor_scalar(out=outsb[:, j * 128:(j + 1) * 128],
                                            in0=pctx[:], scalar1=rzf[:, j:j + 1],
                                            scalar2=None, op0=Alu.mult)
                else:
                    nc.scalar.activation(out=outsb[:, j * 128:(j + 1) * 128],
                                         in_=pctx[:], func=Act.Copy,
                                         bias=0.0, scale=rzf[:, j:j + 1])
            nc.sync.dma_start(
                out=out_d[n0:n0 + 512, :].rearrange("(j p) o -> p j o", p=128),
                in_=outsb[:].rearrange("p (j o) -> p j o", j=4))

    split_excess_waits(nc)
    return nc


_NC_CACHE = None


def make_in_map(inputs, b0):
    return {
        "x": np.ascontiguousarray(
            inputs["input_context"][b0:b0 + BLOC], np.float32).reshape(NLOC, DI),
        "cand_idx": np.ascontiguousarray(
            inputs["cand_idx"][b0:b0 + BLOC], np.int32).reshape(NLOC, C),
        "cand_mask": np.ascontiguousarray(
            inputs["cand_mask"][b0:b0 + BLOC]).astype(np.uint8).reshape(NLOC, C),
        "W": np.ascontiguousarray(inputs["W"], np.float32),
        "pos_embed": np.ascontiguousarray(inputs["pos_embed"], np.float32),
        "word_seq_len": np.ascontiguousarray(
            inputs["word_seq_len"][b0:b0 + BLOC], np.int32).reshape(1, BLOC),
    }


def kernel(**inputs):
    global _NC_CACHE
    from concourse.bass_utils import run_bass_kernel_spmd

    if _NC_CACHE is None:
        _NC_CACHE = build_kernel()
    nc = _NC_CACHE

    in_maps = [make_in_map(inputs, c * BLOC) for c in range(NCORES)]
    res = run_bass_kernel_spmd(nc, in_maps, core_ids=list(range(NCORES)))
    out = np.empty((B, L, DO), np.float32)
    for c in range(NCORES):
        out[c * BLOC:(c + 1) * BLOC] = res.results[c]["out"].reshape(BLOC, L, DO)
    return out

